# revision 11
# baseline (speedup 1.0000x reference)
"""AccRNNCell Trainium2 kernel — 8-core data-parallel over batch.

Layout: everything transposed ([feature, batch] on device) so matmul outputs
land directly in the layout the next matmul consumes — zero on-device
transposes. Weights are the stationary operand (lhsT = W as stored, [K, M]);
activations are the moving operand [K<=128, BL=64]. bf16 matmul inputs, f32
PSUM accumulation, f32 running accumulator and outputs.

Algebraic restructure (all fusions precomputed in f32 on host):
    s0(t) = s0(t-1)@WA0 + x(t)@WB0x + acc(t-2)@WB0a + s2(t-1)@Wfuse0
    s1(t) = s1(t-1)@WA1 + s0(t)@WF01
    s2(t) = s2(t-1)@WA2 + s1(t)@WF12
    y(t)  = s2(t)@WFy ;  acc(t) = acc(t-1) + y(t)      (off critical path)
where Wfuse0 = WC2@Wout@WB0a, WF01 = WC0@WBr0, WF12 = WC1@WBr1,
WFy = WC2@Wout. The C-stage disappears; the per-step chain is 3 matmul
groups + 3 PSUM->SBUF copies. Biases are all zero for this problem
(asserted on host).

PSUM discipline: matmul start=True clears the WHOLE bank, so each
accumulation group owns a private [128, 2, 512] (2-bank) tile; the m-pair
shares one DVE copy.

Host side: pre-transpose x to [F, T, BL] per core, cast weights/x to bf16;
output returns [P, T, BL] f32 per core, host reassembles [B, T, P].
"""

import numpy as np
import ml_dtypes

import concourse.bass as bass
from concourse import bacc
import concourse.mybir as mybir
import concourse.tile as tile
from concourse.bass import ds
from concourse.bass_utils import run_bass_kernel_spmd

L = 3
U = 512
P = 32
F = 64
B = 512
T = 512
NCORES = 8
BL = B // NCORES          # batch rows per core = 64
UK = U // 128             # 4 k/m chunks of 128
UNROLL = 32               # timesteps per For_i body

BF16 = mybir.dt.bfloat16
F32 = mybir.dt.float32
ADD = mybir.AluOpType.add


def build_graph(t_steps=T, unroll=UNROLL, debug=False, static=False):
    """Build the single-core Bass graph (same graph runs SPMD on 8 cores)."""
    assert t_steps % unroll == 0
    assert unroll % 2 == 0, "acc ping-pong needs even unroll"
    nblk = t_steps // unroll
    nc = bacc.Bacc()

    x_d = nc.declare_dram_parameter("xT", [F, nblk, unroll, BL], BF16, isOutput=False)
    wa_d = nc.declare_dram_parameter("wa", [L, UK, 128, U], BF16, isOutput=False)
    wb0x_d = nc.declare_dram_parameter("wb0x", [F, U], BF16, isOutput=False)
    wb0a_d = nc.declare_dram_parameter("wb0a", [P, U], BF16, isOutput=False)
    wf01_d = nc.declare_dram_parameter("wf01", [UK, 128, U], BF16, isOutput=False)
    wf12_d = nc.declare_dram_parameter("wf12", [UK, 128, U], BF16, isOutput=False)
    wfu0_d = nc.declare_dram_parameter("wfu0", [UK, 128, U], BF16, isOutput=False)
    wfy_d = nc.declare_dram_parameter("wfy", [UK, 128, P], BF16, isOutput=False)
    y_d = nc.declare_dram_parameter("yT", [P, nblk, unroll, BL], F32, isOutput=True)
    dbg_d = None
    if debug:
        dbg_d = nc.declare_dram_parameter("dbg", [L, 128, UK, BL], BF16, isOutput=True)

    with tile.TileContext(nc) as tc:
        with (
            tc.tile_pool(name="const", bufs=1) as cpool,
            tc.tile_pool(name="state", bufs=1) as stpool,
            tc.tile_pool(name="accbf", bufs=2) as abpool,
            tc.tile_pool(name="ystage", bufs=2) as ypool,
            tc.tile_pool(name="ps_s", bufs=3, space="PSUM") as spool,
            tc.tile_pool(name="ps_y", bufs=2, space="PSUM") as yppool,
        ):
            # ---- load weights + x into SBUF once ----
            wa_sb = [cpool.tile([128, UK, U], BF16, tag=f"wa{l}", name=f"wa{l}") for l in range(L)]
            wf01_sb = cpool.tile([128, UK, U], BF16, tag="wf01")
            wf12_sb = cpool.tile([128, UK, U], BF16, tag="wf12")
            wfu0_sb = cpool.tile([128, UK, U], BF16, tag="wfu0")
            wfy_sb = cpool.tile([128, UK, P], BF16, tag="wfy")
            wb0x_sb = cpool.tile([F, U], BF16, tag="wb0x")
            wb0a_sb = cpool.tile([P, U], BF16, tag="wb0a")
            x_sb = cpool.tile([F, nblk, unroll, BL], BF16, tag="x")

            for k in range(UK):
                for l in range(L):
                    nc.sync.dma_start(out=wa_sb[l][:, k, :], in_=wa_d[l, k])
                nc.sync.dma_start(out=wf01_sb[:, k, :], in_=wf01_d[k])
                nc.sync.dma_start(out=wf12_sb[:, k, :], in_=wf12_d[k])
                nc.sync.dma_start(out=wfu0_sb[:, k, :], in_=wfu0_d[k])
                nc.sync.dma_start(out=wfy_sb[:, k, :], in_=wfy_d[k])
            nc.sync.dma_start(out=wb0x_sb[:, :], in_=wb0x_d[:, :])
            nc.sync.dma_start(out=wb0a_sb[:, :], in_=wb0a_d[:, :])
            nc.sync.dma_start(out=x_sb[:, :, :, :], in_=x_d[:, :, :, :])

            # ---- persistent state ----
            sT = [stpool.tile([128, UK, BL], BF16, tag=f"sT{l}", name=f"sT{l}") for l in range(L)]
            accE = stpool.tile([P, BL], F32, tag="accE")
            accO = stpool.tile([P, BL], F32, tag="accO")
            for l in range(L):
                nc.vector.memset(sT[l][:, :, :], 0.0)
            nc.vector.memset(accE[:, :], 0.0)
            nc.vector.memset(accO[:, :], 0.0)

            def s_group(l, extra):
                """One state-layer update: A-matmuls + extra(ps, ml, m) into a
                2-bank psum tile per m-pair, then one copy into sT[l]."""
                pss = []
                for mp in range(UK // 2):
                    ps = spool.tile([128, 2, U], F32, tag="ps_s", name="ps_s")
                    pss.append(ps)
                    for ml in range(2):
                        m = 2 * mp + ml
                        for k in range(UK):
                            nc.tensor.matmul(
                                ps[:, ml, 0:BL],
                                wa_sb[l][:, k, ds(m * 128, 128)],
                                sT[l][:, k, :],
                                start=(k == 0),
                                stop=False,
                            )
                        extra(ps, ml, m)
                for mp in range(UK // 2):
                    nc.vector.tensor_copy(
                        out=sT[l][:, 2 * mp:2 * mp + 2, :],
                        in_=pss[mp][:, :, 0:BL],
                    )

            def step(ib, j, y_stage):
                """One timestep: t = ib*unroll + j."""
                par = j % 2
                acc2 = accE if par == 0 else accO   # holds acc(t-2)
                acc1 = accO if par == 0 else accE   # holds acc(t-1)

                acc_bf = abpool.tile([P, BL], BF16, tag="acc_bf")
                nc.vector.tensor_copy(out=acc_bf[:, :], in_=acc2[:, :])

                def s0_extra(ps, ml, m):
                    nc.tensor.matmul(
                        ps[:, ml, 0:BL],
                        wb0x_sb[:, ds(m * 128, 128)],
                        x_sb[:, ds(ib, 1), j, :],
                        start=False, stop=False,
                    )
                    nc.tensor.matmul(
                        ps[:, ml, 0:BL],
                        wb0a_sb[:, ds(m * 128, 128)],
                        acc_bf[:, :],
                        start=False, stop=False,
                    )
                    for k in range(UK):
                        nc.tensor.matmul(
                            ps[:, ml, 0:BL],
                            wfu0_sb[:, k, ds(m * 128, 128)],
                            sT[2][:, k, :],
                            start=False, stop=(k == UK - 1),
                        )

                def mk_f_extra(w_sb, src):
                    def f_extra(ps, ml, m):
                        for k in range(UK):
                            nc.tensor.matmul(
                                ps[:, ml, 0:BL],
                                w_sb[:, k, ds(m * 128, 128)],
                                src[:, k, :],
                                start=False, stop=(k == UK - 1),
                            )
                    return f_extra

                s_group(0, s0_extra)
                s_group(1, mk_f_extra(wf01_sb, sT[0]))
                s_group(2, mk_f_extra(wf12_sb, sT[1]))

                # output head: y = s2 @ WFy (off the recurrence critical path)
                ps_y = yppool.tile([P, BL], F32, tag="ps_y")
                for k in range(UK):
                    nc.tensor.matmul(
                        ps_y[:, :],
                        wfy_sb[:, k, :],
                        sT[2][:, k, :],
                        start=(k == 0),
                        stop=(k == UK - 1),
                    )
                nc.vector.tensor_copy(out=y_stage[:, j, :], in_=ps_y[:, :])
                # acc(t) = acc(t-1) + y(t), overwriting the acc(t-2) slot
                nc.vector.tensor_tensor(
                    out=acc2[:, :],
                    in0=acc1[:, :],
                    in1=y_stage[:, j, :],
                    op=ADD,
                )

            def block_body(ib):
                y_stage = ypool.tile([P, unroll, BL], F32, tag="y_stage")
                for j in range(unroll):
                    step(ib, j, y_stage)
                nc.sync.dma_start(out=y_d[:, ds(ib, 1), :, :], in_=y_stage[:, :, :])

            if nblk == 1 or static:
                for ib in range(nblk):
                    block_body(ib)
            else:
                with tc.For_i(0, nblk, 1, hint_engines=(mybir.EngineType.PE, mybir.EngineType.DVE)) as ib:
                    block_body(ib)
            if debug:
                for l in range(L):
                    nc.sync.dma_start(out=dbg_d[l], in_=sT[l][:, :, :])

    nc.finalize()
    return nc


def _prep_inputs(x, WA, bA, WB0, bB0, WBr, bBr, WC, bC, Wout, bout, t_steps=T, unroll=UNROLL):
    """Host-side shard + transpose + weight fusion + cast. Returns 8 in_maps."""
    for b_ in (bA, bB0, bBr, bC, bout):
        assert np.max(np.abs(np.asarray(b_))) == 0.0, "kernel assumes zero biases"
    bf = ml_dtypes.bfloat16
    nblk = t_steps // unroll
    WA = np.asarray(WA, np.float32)
    WB0 = np.asarray(WB0, np.float32)
    WBr = np.asarray(WBr, np.float32)
    WC = np.asarray(WC, np.float32)
    Wout = np.asarray(Wout, np.float32)

    WFy = WC[2] @ Wout                      # [U, P]
    wmats = {
        "wa": WA.reshape(L, UK, 128, U),
        "wb0x": WB0[:F],
        "wb0a": WB0[F:],
        "wf01": (WC[0] @ WBr[0]).reshape(UK, 128, U),
        "wf12": (WC[1] @ WBr[1]).reshape(UK, 128, U),
        "wfu0": (WFy @ WB0[F:]).reshape(UK, 128, U),
        "wfy": WFy.reshape(UK, 128, P),
    }
    wmats = {k: np.ascontiguousarray(v).astype(bf) for k, v in wmats.items()}

    in_maps = []
    for c in range(NCORES):
        xs = x[c * BL:(c + 1) * BL, :t_steps, :]          # [BL, t, F]
        xT = np.ascontiguousarray(xs.transpose(2, 1, 0))  # [F, t, BL]
        xT = xT.reshape(F, nblk, unroll, BL).astype(bf)
        in_maps.append({"xT": xT, **wmats})
    return in_maps


def _gather_output(results, t_steps=T):
    """results[i]['yT'] [P, nblk, unroll, BL] -> full y [B, t, P] f32."""
    outs = []
    for c in range(NCORES):
        yT = np.asarray(results[c]["yT"], dtype=np.float32).reshape(P, t_steps, BL)
        outs.append(np.ascontiguousarray(yT.transpose(2, 1, 0)))  # [BL, t, P]
    return np.concatenate(outs, axis=0)


def kernel(x, WA, bA, WB0, bB0, WBr, bBr, WC, bC, Wout, bout):
    nc = build_graph(T, UNROLL)
    in_maps = _prep_inputs(x, WA, bA, WB0, bB0, WBr, bBr, WC, bC, Wout, bout)
    res = run_bass_kernel_spmd(nc, in_maps, core_ids=list(range(NCORES)))
    return _gather_output(res.results)


# revision 14
# speedup vs baseline: 1.0714x; 1.0714x over previous
"""AccRNNCell Trainium2 kernel — 8-core data-parallel over batch.

Layout: everything transposed ([feature, batch] on device); weights are the
stationary operand (lhsT = W as stored, [K, M]); activations are the moving
operand [K<=128, BL=64]. bf16 matmul inputs, f32 PSUM accumulation, f32
running accumulator and outputs.

K-step expanded recurrence (K=4). With S = [s0 s1 s2] (3U = 1536) the
original per-step network is linear:
    S(t) = S(t-1)@M + x(t)@Mx + acc(t-2)@Ma ;  y(t) = S(t)@Wy
(acc enters with a 2-step delay via the exact identity
 acc(t-1)@WB0a = acc(t-2)@WB0a + s2(t-1)@(WC2@Wout@WB0a)).
Expanding k steps and packing the k x-vectors / k delayed-acc vectors into
single contraction operands:
    S(t0+K-1) = S(t0-1)@M^K + xcat@Wxcat + acccat@Wacat
    y(t0+i)   = S(t0-1)@WyS[i] + xcat@Wyxcat[i] + acccat[:used]@Wyacat[i]
All fusion products are computed in float64 on the host and cast to bf16
once. Per 4 steps: 238 matmuls (~60/step vs 108 unfused) and a single
S handoff. The y/acc chain is emitted first each macro so the scheduler
fills its semaphore waits with the big S-update matmuls.

PSUM discipline: matmul start=True clears the WHOLE bank, so each
accumulation group owns a private tile; S m-chunk pairs share a 2-bank tile
and one DVE copy.

Biases are all zero for this problem (asserted on host).
"""

import numpy as np
import ml_dtypes

import concourse.bass as bass
from concourse import bacc
import concourse.mybir as mybir
import concourse.tile as tile
from concourse.bass import ds
from concourse.bass_utils import run_bass_kernel_spmd

L = 3
U = 512
P = 32
F = 64
B = 512
T = 512
NCORES = 8
BL = B // NCORES          # batch rows per core = 64
SU = 3 * U                # stacked state width 1536
SK = SU // 128            # 12 chunks
KST = 4                   # macro-step: timesteps folded into one linear map
UNROLL = 32               # timesteps per For_i body (8 macros)

BF16 = mybir.dt.bfloat16
F32 = mybir.dt.float32
ADD = mybir.AluOpType.add


def build_graph(t_steps=T, unroll=UNROLL, static=False):
    """Build the single-core Bass graph (same graph runs SPMD on 8 cores)."""
    assert t_steps % unroll == 0 and unroll % KST == 0
    nblk = t_steps // unroll
    nmac = unroll // KST          # macros per body
    nc = bacc.Bacc()

    # x pair-stacked: rows 0:64 = x(even t), 64:128 = x(odd t)
    x_d = nc.declare_dram_parameter("xT2", [2 * F, nblk, unroll // 2, BL], BF16, isOutput=False)
    wS_d = nc.declare_dram_parameter("wS", [SK, 128, SU], BF16, isOutput=False)
    wxc_d = nc.declare_dram_parameter("wxc", [2, 128, SU], BF16, isOutput=False)
    waclo_d = nc.declare_dram_parameter("waclo", [2 * P, SU], BF16, isOutput=False)
    wac0_d = nc.declare_dram_parameter("wac0", [P, SU], BF16, isOutput=False)
    wac1_d = nc.declare_dram_parameter("wac1", [P, SU], BF16, isOutput=False)
    wys_d = nc.declare_dram_parameter("wys", [KST, SK, 128, P], BF16, isOutput=False)
    wyx_d = nc.declare_dram_parameter("wyx", [KST, 2, 128, P], BF16, isOutput=False)
    wyalo_d = nc.declare_dram_parameter("wyalo", [KST, 2 * P, P], BF16, isOutput=False)
    wya0_d = nc.declare_dram_parameter("wya0", [KST, P, P], BF16, isOutput=False)
    wya1_d = nc.declare_dram_parameter("wya1", [KST, P, P], BF16, isOutput=False)
    y_d = nc.declare_dram_parameter("yT", [P, nblk, unroll, BL], F32, isOutput=True)

    with tile.TileContext(nc) as tc:
        with (
            tc.tile_pool(name="const", bufs=1) as cpool,
            tc.tile_pool(name="state", bufs=1) as stpool,
            tc.tile_pool(name="acat", bufs=2) as acpool,
            tc.tile_pool(name="ystage", bufs=2) as ypool,
            tc.tile_pool(name="ps_s", bufs=3, space="PSUM") as spool,
            tc.tile_pool(name="ps_y", bufs=2, space="PSUM") as yppool,
        ):
            # ---- load weights + x into SBUF once ----
            wS_sb = cpool.tile([128, SK, SU], BF16, tag="wS")
            wxc_sb = cpool.tile([128, 2, SU], BF16, tag="wxc")
            waclo_sb = cpool.tile([2 * P, SU], BF16, tag="waclo")
            wac0_sb = cpool.tile([P, SU], BF16, tag="wac0")
            wac1_sb = cpool.tile([P, SU], BF16, tag="wac1")
            wys_sb = cpool.tile([128, KST, SK, P], BF16, tag="wys")
            wyx_sb = cpool.tile([128, KST, 2, P], BF16, tag="wyx")
            wyalo_sb = cpool.tile([2 * P, KST, P], BF16, tag="wyalo")
            wya0_sb = cpool.tile([P, KST, P], BF16, tag="wya0")
            wya1_sb = cpool.tile([P, KST, P], BF16, tag="wya1")
            x_sb = cpool.tile([2 * F, nblk, unroll // 2, BL], BF16, tag="x")

            for k in range(SK):
                nc.sync.dma_start(out=wS_sb[:, k, :], in_=wS_d[k])
            for c in range(2):
                nc.sync.dma_start(out=wxc_sb[:, c, :], in_=wxc_d[c])
            nc.sync.dma_start(out=waclo_sb[:, :], in_=waclo_d[:, :])
            nc.sync.dma_start(out=wac0_sb[:, :], in_=wac0_d[:, :])
            nc.sync.dma_start(out=wac1_sb[:, :], in_=wac1_d[:, :])
            for i in range(KST):
                for k in range(SK):
                    nc.sync.dma_start(out=wys_sb[:, i, k, :], in_=wys_d[i, k])
                for c in range(2):
                    nc.sync.dma_start(out=wyx_sb[:, i, c, :], in_=wyx_d[i, c])
                nc.sync.dma_start(out=wyalo_sb[:, i, :], in_=wyalo_d[i])
                nc.sync.dma_start(out=wya0_sb[:, i, :], in_=wya0_d[i])
                nc.sync.dma_start(out=wya1_sb[:, i, :], in_=wya1_d[i])
            nc.sync.dma_start(out=x_sb[:, :, :, :], in_=x_d[:, :, :, :])

            # ---- persistent state (S double-buffered per macro) ----
            S_ping = stpool.tile([128, SK, BL], BF16, tag="S_ping")
            S_pong = stpool.tile([128, SK, BL], BF16, tag="S_pong")
            accR = [stpool.tile([P, BL], F32, tag=f"accR{r}", name=f"accR{r}") for r in range(4)]
            nc.vector.memset(S_ping[:, :, :], 0.0)
            for r in range(4):
                nc.vector.memset(accR[r][:, :], 0.0)

            def macro(ib, mi, y_stage):
                """Four timesteps t0..t0+3, t0 = (ib*unroll) + 4*mi."""
                S_src = S_ping if mi % 2 == 0 else S_pong
                S_dst = S_pong if mi % 2 == 0 else S_ping
                # delayed-acc operands: acat_lo = [acc(t0-2); acc(t0-1)]
                # (both known at macro start); acc0/acc1 written mid-chain.
                acat_lo = acpool.tile([2 * P, BL], BF16, tag="acat_lo")
                nc.vector.tensor_copy(out=acat_lo[0:P, :], in_=accR[2][:, :])
                nc.vector.tensor_copy(out=acat_lo[P:2 * P, :], in_=accR[3][:, :])
                acc0_bf = acpool.tile([P, BL], BF16, tag="acc0_bf")
                acc1_bf = acpool.tile([P, BL], BF16, tag="acc1_bf")

                # ---- y/acc chain (emitted first: highest scheduler priority,
                # its waits get filled by the S-update matmuls below) ----
                for i in range(KST):
                    ps_y = yppool.tile([P, BL], F32, tag="ps_y")
                    for k in range(SK):
                        nc.tensor.matmul(
                            ps_y[:, :],
                            wys_sb[:, i, k, :],
                            S_src[:, k, :],
                            start=(k == 0), stop=False,
                        )
                    for c in range(i // 2 + 1):
                        nc.tensor.matmul(
                            ps_y[:, :],
                            wyx_sb[:, i, c, :],
                            x_sb[:, ds(ib, 1), 2 * mi + c, :],
                            start=False, stop=False,
                        )
                    nc.tensor.matmul(
                        ps_y[:, :],
                        wyalo_sb[:, i, :],
                        acat_lo[:, :],
                        start=False, stop=(i < 2),
                    )
                    if i >= 2:
                        nc.tensor.matmul(
                            ps_y[:, :],
                            wya0_sb[:, i, :],
                            acc0_bf[:, :],
                            start=False, stop=(i == 2),
                        )
                    if i == 3:
                        nc.tensor.matmul(
                            ps_y[:, :],
                            wya1_sb[:, i, :],
                            acc1_bf[:, :],
                            start=False, stop=True,
                        )
                    nc.vector.tensor_copy(out=y_stage[:, KST * mi + i, :], in_=ps_y[:, :])
                    # acc(t0+i) = acc(t0+i-1) + y ; ring slot i
                    nc.vector.tensor_tensor(
                        out=accR[i][:, :],
                        in0=accR[(i - 1) % 4][:, :],
                        in1=ps_y[:, :],
                        op=ADD,
                    )
                    if i == 0:
                        nc.vector.tensor_copy(out=acc0_bf[:, :], in_=accR[0][:, :])
                    elif i == 1:
                        nc.vector.tensor_copy(out=acc1_bf[:, :], in_=accR[1][:, :])

                # ---- S update: S(t0+3) = S(t0-1)@M^4 + xcat@Wxcat + acccat@Wacat ----
                pss = []
                for mp in range(SK // 2):
                    ps = spool.tile([128, 2, U], F32, tag="ps_s", name="ps_s")
                    pss.append(ps)
                    for ml in range(2):
                        m = 2 * mp + ml
                        for k in range(SK):
                            nc.tensor.matmul(
                                ps[:, ml, 0:BL],
                                wS_sb[:, k, ds(m * 128, 128)],
                                S_src[:, k, :],
                                start=(k == 0), stop=False,
                            )
                        for c in range(2):
                            nc.tensor.matmul(
                                ps[:, ml, 0:BL],
                                wxc_sb[:, c, ds(m * 128, 128)],
                                x_sb[:, ds(ib, 1), 2 * mi + c, :],
                                start=False, stop=False,
                            )
                        nc.tensor.matmul(
                            ps[:, ml, 0:BL],
                            waclo_sb[:, ds(m * 128, 128)],
                            acat_lo[:, :],
                            start=False, stop=False,
                        )
                        nc.tensor.matmul(
                            ps[:, ml, 0:BL],
                            wac0_sb[:, ds(m * 128, 128)],
                            acc0_bf[:, :],
                            start=False, stop=False,
                        )
                        nc.tensor.matmul(
                            ps[:, ml, 0:BL],
                            wac1_sb[:, ds(m * 128, 128)],
                            acc1_bf[:, :],
                            start=False, stop=True,
                        )
                for mp in range(SK // 2):
                    nc.vector.tensor_copy(
                        out=S_dst[:, 2 * mp:2 * mp + 2, :],
                        in_=pss[mp][:, :, 0:BL],
                    )

            def block_body(ib):
                y_stage = ypool.tile([P, unroll, BL], F32, tag="y_stage")
                for mi in range(nmac):
                    macro(ib, mi, y_stage)
                nc.sync.dma_start(out=y_d[:, ds(ib, 1), :, :], in_=y_stage[:, :, :])

            if nblk == 1 or static:
                for ib in range(nblk):
                    block_body(ib)
            else:
                with tc.For_i(0, nblk, 1, hint_engines=(mybir.EngineType.PE, mybir.EngineType.DVE)) as ib:
                    block_body(ib)

    nc.finalize()
    return nc


def _fuse_weights(WA, WB0, WBr, WC, Wout, K=KST):
    """Host-side f64 fusion products for the K-step expanded recurrence."""
    f8 = np.float64
    WB0x, WB0a = WB0[:F].astype(f8), WB0[F:].astype(f8)
    WF01 = WC[0].astype(f8) @ WBr[0].astype(f8)
    WF12 = WC[1].astype(f8) @ WBr[1].astype(f8)
    WFy = WC[2].astype(f8) @ Wout.astype(f8)
    Wfu0 = WFy @ WB0a
    A0, A1, A2 = (WA[i].astype(f8) for i in range(3))
    Z = np.zeros((U, U), f8)
    M = np.block([
        [A0,   A0 @ WF01,   A0 @ WF01 @ WF12],
        [Z,    A1,          A1 @ WF12],
        [Wfu0, Wfu0 @ WF01, A2 + Wfu0 @ WF01 @ WF12]])
    Mx = np.hstack([WB0x, WB0x @ WF01, WB0x @ WF01 @ WF12])   # [F, 3U]
    Ma = np.hstack([WB0a, WB0a @ WF01, WB0a @ WF01 @ WF12])   # [P, 3U]
    Wy = np.vstack([np.zeros((U, P)), np.zeros((U, P)), WFy])  # [3U, P]

    Mp = [np.linalg.matrix_power(M, j) for j in range(K + 1)]
    WS = Mp[K]
    Wxcat = np.vstack([Mx @ Mp[K - 1 - i] for i in range(K)])   # [K*F, 3U]
    Wacat = np.vstack([Ma @ Mp[K - 1 - i] for i in range(K)])   # [K*P, 3U]
    WyS = [Mp[i + 1] @ Wy for i in range(K)]                    # [3U, P]
    Wyx = [Mx @ Mp[j] @ Wy for j in range(K)]
    Wya = [Ma @ Mp[j] @ Wy for j in range(K)]
    Wyxcat = [np.vstack([Wyx[i - j] if j <= i else np.zeros((F, P)) for j in range(K)])
              for i in range(K)]
    Wyacat = [np.vstack([Wya[i - j] if j <= i else np.zeros((P, P)) for j in range(K)])
              for i in range(K)]
    return WS, Wxcat, Wacat, WyS, Wyxcat, Wyacat


def _prep_inputs(x, WA, bA, WB0, bB0, WBr, bBr, WC, bC, Wout, bout, t_steps=T, unroll=UNROLL):
    """Host-side shard + transpose + weight fusion + cast. Returns 8 in_maps."""
    for b_ in (bA, bB0, bBr, bC, bout):
        assert np.max(np.abs(np.asarray(b_))) == 0.0, "kernel assumes zero biases"
    bf = ml_dtypes.bfloat16
    nblk = t_steps // unroll
    WA, WB0, WBr = np.asarray(WA, np.float32), np.asarray(WB0, np.float32), np.asarray(WBr, np.float32)
    WC, Wout = np.asarray(WC, np.float32), np.asarray(Wout, np.float32)

    WS, Wxcat, Wacat, WyS, Wyxcat, Wyacat = _fuse_weights(WA, WB0, WBr, WC, Wout)
    wmats = {
        "wS": WS.reshape(SK, 128, SU),
        "wxc": Wxcat.reshape(2, 128, SU),
        "waclo": Wacat[0:2 * P],
        "wac0": Wacat[2 * P:3 * P],
        "wac1": Wacat[3 * P:4 * P],
        "wys": np.stack([w.reshape(SK, 128, P) for w in WyS]),
        "wyx": np.stack([w.reshape(2, 128, P) for w in Wyxcat]),
        "wyalo": np.stack([w[0:2 * P] for w in Wyacat]),
        "wya0": np.stack([w[2 * P:3 * P] for w in Wyacat]),
        "wya1": np.stack([w[3 * P:4 * P] for w in Wyacat]),
    }
    wmats = {k: np.ascontiguousarray(v).astype(bf) for k, v in wmats.items()}

    in_maps = []
    for c in range(NCORES):
        xs = x[c * BL:(c + 1) * BL, :t_steps, :]              # [BL, t, F]
        # pair-stack: [2F, t/2, BL] with rows 0:64 = x(2m), 64:128 = x(2m+1)
        xp = xs.reshape(BL, t_steps // 2, 2, F).transpose(2, 3, 1, 0)  # [2, F, t/2, BL]
        xp = xp.reshape(2 * F, nblk, unroll // 2, BL)
        in_maps.append({"xT2": np.ascontiguousarray(xp).astype(bf), **wmats})
    return in_maps


def _gather_output(results, t_steps=T):
    """results[i]['yT'] [P, nblk, unroll, BL] -> full y [B, t, P] f32."""
    outs = []
    for c in range(NCORES):
        yT = np.asarray(results[c]["yT"], dtype=np.float32).reshape(P, t_steps, BL)
        outs.append(np.ascontiguousarray(yT.transpose(2, 1, 0)))  # [BL, t, P]
    return np.concatenate(outs, axis=0)


def kernel(x, WA, bA, WB0, bB0, WBr, bBr, WC, bC, Wout, bout):
    nc = build_graph(T, UNROLL, static=True)
    in_maps = _prep_inputs(x, WA, bA, WB0, bB0, WBr, bBr, WC, bC, Wout, bout)
    res = run_bass_kernel_spmd(nc, in_maps, core_ids=list(range(NCORES)))
    return _gather_output(res.results)
